# revision 10
# baseline (speedup 1.0000x reference)
"""Disentangled multi-head attention on 8 trn2 NeuronCores.

Sharding: core c -> (batch b = c//2, head-group g = c%2). Each core owns one
batch element and 8 of the 16 heads. Host pre-transposes x/pos and the weight
slices so every on-chip matmul operand is already in [K-on-partitions] layout.

Math (per core, 8 local heads, hd=64):
  qT, kT  = Wcq/Wck slices applied to x      (transposed layout [feat, T])
  kpT     = kT + (Wp slice applied to pos)   (scores = q@(k+pk)^T = q@kp^T)
  S_h     = q_h @ kp_h^T        -> exp(S*scale) -> rowsums (ACT accum) -> attn
  ST_h    = kp_h @ q_h^T        -> exp -> O_h^T = (k_h^T @ expST) * recip
  yT      = Wo_slice^T-matmul over O^T  (host: out[b] = yT0.T + yT1.T + bo)
Outputs per core: attn [8, T, T], yT [D, T].
"""

import numpy as np

B, T, D, H = 4, 1024, 1024, 16
HD = D // H            # 64 head dim
NCORES = 8
GH = H // 2            # 8 heads per core
DL = GH * HD           # 512 local feature dim
P = 128
KD = D // P            # 8 K-subtiles over D
MT = T // P            # 8 token tiles
FQ = DL // P           # 4 local-feature subtiles
NSP = 512              # matmul moving free dim (one fp32 PSUM bank)
NCH = T // NSP         # 2
SCALE = HD ** -0.5

TRACE = False
LAST_RESULTS = None
_NC_CACHE = {}


def build_bass():
    from contextlib import ExitStack

    import concourse.mybir as mybir
    import concourse.tile as tile
    from concourse import bacc
    from concourse.masks import make_identity

    f32 = mybir.dt.float32
    f32r = mybir.dt.float32r
    AF = mybir.ActivationFunctionType
    OP = mybir.AluOpType

    nc = bacc.Bacc("TRN2")

    xT = nc.dram_tensor("xT", [D, T], f32r, kind="ExternalInput")
    posT = nc.dram_tensor("posT", [D, T], f32r, kind="ExternalInput")
    wcqT = nc.dram_tensor("wcqT", [D, DL], f32r, kind="ExternalInput")
    wckT = nc.dram_tensor("wckT", [D, DL], f32r, kind="ExternalInput")
    wpT = nc.dram_tensor("wpT", [D, DL], f32r, kind="ExternalInput")
    woT = nc.dram_tensor("woT", [DL, D], f32r, kind="ExternalInput")
    bcq = nc.dram_tensor("bcq", [DL], f32, kind="ExternalInput")
    bck = nc.dram_tensor("bck", [DL], f32, kind="ExternalInput")
    bp = nc.dram_tensor("bp", [DL], f32, kind="ExternalInput")
    attn_out = nc.dram_tensor("attn_out", [GH, T, T], f32, kind="ExternalOutput")
    yT_out = nc.dram_tensor("yT_out", [D, T], f32, kind="ExternalOutput")

    with tile.TileContext(nc) as tc, ExitStack() as ctx:
        persist = ctx.enter_context(tc.tile_pool(name="persist", bufs=1))
        qT = persist.tile([P, FQ, T], f32r)       # [feat%128, feat//128, T]
        kT = persist.tile([P, FQ, T], f32)
        kpT = persist.tile([P, FQ, T], f32r)
        k_nat = persist.tile([P, MT, DL], f32r)   # [t%128, t//128, feat]
        woT_sb = persist.tile([64, GH, D], f32r)  # [dl%64, head, dout]
        bias_cols = persist.tile([P, 3, FQ], f32)
        rowsum = persist.tile([P, GH, MT], f32)
        recip = persist.tile([P, GH, MT], f32)
        ident = persist.tile([P, P], f32)

        nc.sync.dma_start(bias_cols[:, 0], bcq.rearrange("(j p) -> p j", p=P))
        nc.sync.dma_start(bias_cols[:, 1], bck.rearrange("(j p) -> p j", p=P))
        nc.sync.dma_start(bias_cols[:, 2], bp.rearrange("(j p) -> p j", p=P))
        make_identity(nc, ident)

        dramp = ctx.enter_context(tc.tile_pool(name="dramp", bufs=1, space="DRAM"))
        recip_dram = dramp.tile([GH, T], f32)

        xT_v = xT.rearrange("(ks p) t -> p ks t", p=P)
        posT_v = posT.rearrange("(ks p) t -> p ks t", p=P)

        # ---- Phase A: projections (transposed layouts) ----
        with tc.tile_pool(name="wx", bufs=1) as wx, \
             tc.tile_pool(name="rhsp", bufs=3) as rhsp, \
             tc.tile_pool(name="pps", bufs=2, space="PSUM") as pps:
            wcq_sb = wx.tile([P, KD, DL], f32r)
            wck_sb = wx.tile([P, KD, DL], f32r)
            wp_sb = wx.tile([P, KD, DL], f32r)
            wcq_v = wcqT.rearrange("(ks p) f -> p ks f", p=P)
            wck_v = wckT.rearrange("(ks p) f -> p ks f", p=P)
            wp_v = wpT.rearrange("(ks p) f -> p ks f", p=P)

            # per-ks chunked loads so matmuls start after the first chunk
            for ks in range(KD):
                nc.sync.dma_start(wck_sb[:, ks], wck_v[:, ks])

            def proj(w_sb, dst_evac, xc):
                for j in range(FQ):
                    jsl = slice(j * P, (j + 1) * P)
                    ps = pps.tile([P, NSP], f32, tag="pp", name=f"pp{j}")
                    for ks in range(KD):
                        nc.tensor.matmul(
                            ps,
                            lhsT=w_sb[:, ks, jsl],
                            rhs=xc[:, ks],
                            start=(ks == 0), stop=(ks == KD - 1),
                        )
                    dst_evac(j, ps)

            def evac_q(j, ps, nsl):
                nc.vector.tensor_scalar_add(
                    qT[:, j, nsl], ps, bias_cols[:, 0, j:j + 1])

            def evac_k(j, ps, nsl):
                nc.vector.tensor_scalar_add(
                    kT[:, j, nsl], ps, bias_cols[:, 1, j:j + 1])

            def evac_kp(j, ps, nsl):
                # kpT = (pk + bp) + kT
                nc.vector.scalar_tensor_tensor(
                    out=kpT[:, j, nsl], in0=ps,
                    scalar=bias_cols[:, 2, j:j + 1], in1=kT[:, j, nsl],
                    op0=OP.add, op1=OP.add,
                )

            from functools import partial

            xcs = []
            for n in range(NCH):
                nsl = slice(n * NSP, (n + 1) * NSP)
                xc = rhsp.tile([P, KD, NSP], f32r, tag=f"x{n}", bufs=1,
                               name=f"xc{n}")
                for ks in range(KD):
                    nc.sync.dma_start(xc[:, ks], xT_v[:, ks, nsl])
                xcs.append(xc)
                proj(wck_sb, partial(evac_k, nsl=nsl), xc)
                if n == 0:
                    for ks in range(KD):
                        nc.sync.dma_start(wp_sb[:, ks], wp_v[:, ks])
                pc = rhsp.tile([P, KD, NSP], f32r, tag="pc", bufs=1, name="pc")
                for ks in range(KD):
                    nc.sync.dma_start(pc[:, ks], posT_v[:, ks, nsl])
                proj(wp_sb, partial(evac_kp, nsl=nsl), pc)
                if n == 0:
                    for ks in range(KD):
                        nc.sync.dma_start(wcq_sb[:, ks], wcq_v[:, ks])
            for n in range(NCH):
                nsl = slice(n * NSP, (n + 1) * NSP)
                proj(wcq_sb, partial(evac_q, nsl=nsl), xcs[n])

        # ---- Phase B/C/D ----
        with tc.tile_pool(name="mid", bufs=1) as mid, \
             tc.tile_pool(name="attp", bufs=4) as attp, \
             tc.tile_pool(name="estp", bufs=10) as estp, \
             tc.tile_pool(name="rbc", bufs=2) as rbc, \
             tc.tile_pool(name="smal", bufs=2) as smal, \
             tc.tile_pool(name="ysbp", bufs=3) as ysbp, \
             tc.tile_pool(name="score", bufs=3, space="PSUM") as score, \
             tc.tile_pool(name="ob", bufs=2, space="PSUM") as ob:
            oT = mid.tile([64, GH, T], f32r)
            nc.sync.dma_start(woT_sb, woT.rearrange("(h d) f -> d h f", d=64))

            # k natural layout via PE transposes of kT
            for ts in range(MT):
                for fj in range(FQ):
                    tpt = score.tile([P, T], f32, tag="sc", name="tpt")
                    nc.tensor.transpose(
                        tpt[:, 0:P], kT[:, fj, ts * P:(ts + 1) * P], ident)
                    nc.vector.tensor_copy(
                        out=k_nat[:, ts, fj * P:(fj + 1) * P], in_=tpt[:, 0:P]
                    )

            for h in range(GH):
                base = 64 * (h % 2)
                j = h // 2
                qh = qT[base:base + 64, j]     # [64, T]
                kph = kpT[base:base + 64, j]   # [64, T]

                # S path: attn output + rowsums
                for qt in range(MT):
                    ps = score.tile([P, T], f32, tag="sc")
                    for n in range(NCH):
                        nc.tensor.matmul(
                            ps[:, n * NSP:(n + 1) * NSP],
                            lhsT=qh[:, qt * P:(qt + 1) * P],
                            rhs=kph[:, n * NSP:(n + 1) * NSP],
                            start=True, stop=True,
                        )
                    att = attp.tile([P, T], f32, tag="att")
                    nc.scalar.activation(
                        att, ps, AF.Exp, scale=SCALE,
                        accum_out=rowsum[:, h, qt:qt + 1],
                    )
                    nc.vector.reciprocal(
                        recip[:, h, qt:qt + 1], rowsum[:, h, qt:qt + 1]
                    )
                    nc.vector.tensor_scalar_mul(att, att, recip[:, h, qt:qt + 1])
                    nc.sync.dma_start(attn_out[h, qt * P:(qt + 1) * P, :], att)

                # recip row [1, T] via strided DMA to DRAM, then broadcast back
                nc.sync.dma_start(
                    recip_dram[h].rearrange("(a b) -> b a", a=MT),
                    recip[:, h, :],
                )
                rb = rbc.tile([P, T], f32, tag="rb")
                nc.sync.dma_start(rb, recip_dram[h][None, :].to_broadcast((P, T)))

                # ST path + O accumulation
                po = [ob.tile([64, NSP], f32, tag="ob", name=f"po{n}_{h}")
                      for n in range(NCH)]
                ests = []
                for kt in range(MT):
                    ps2 = score.tile([P, T], f32, tag="sc")
                    for n in range(NCH):
                        nc.tensor.matmul(
                            ps2[:, n * NSP:(n + 1) * NSP],
                            lhsT=kph[:, kt * P:(kt + 1) * P],
                            rhs=qh[:, n * NSP:(n + 1) * NSP],
                            start=True, stop=True,
                        )
                    est = estp.tile([P, T], f32r, tag="est")
                    nc.scalar.activation(est, ps2, AF.Exp, scale=SCALE)
                    ests.append(est)
                for kt in range(MT):
                    for n in range(NCH):
                        nc.tensor.matmul(
                            po[n],
                            lhsT=k_nat[:, kt, h * HD:(h + 1) * HD],
                            rhs=ests[kt][:, n * NSP:(n + 1) * NSP],
                            start=(kt == 0), stop=(kt == MT - 1),
                        )
                for n in range(NCH):
                    nsl = slice(n * NSP, (n + 1) * NSP)
                    nc.vector.tensor_tensor(
                        out=oT[:, h, nsl],
                        in0=po[n],
                        in1=rb[0:64, nsl],
                        op=OP.mult,
                    )

            # yT = Wo_slice^T-matmul over O^T
            for m in range(KD):
                for n in range(NCH):
                    nsl = slice(n * NSP, (n + 1) * NSP)
                    py = ob.tile([P, NSP], f32, tag="ob")
                    for hh in range(GH):
                        nc.tensor.matmul(
                            py,
                            lhsT=woT_sb[:, hh, m * P:(m + 1) * P],
                            rhs=oT[:, hh, nsl],
                            start=(hh == 0), stop=(hh == GH - 1),
                        )
                    ys = ysbp.tile([P, NSP], f32, tag="ys")
                    nc.vector.tensor_copy(out=ys, in_=py)
                    nc.sync.dma_start(yT_out[m * P:(m + 1) * P, nsl], ys)

    nc.finalize()
    return nc


def make_in_maps(x, pos, Wc, bc, Wp, bp, Wo):
    in_maps = []
    for c in range(NCORES):
        b, g = divmod(c, 2)
        sl = slice(g * DL, (g + 1) * DL)
        ksl = slice(D + g * DL, D + (g + 1) * DL)
        in_maps.append({
            "xT": np.ascontiguousarray(x[b].T),
            "posT": np.ascontiguousarray(pos[b].T),
            "wcqT": np.ascontiguousarray(Wc[sl, :].T),
            "wckT": np.ascontiguousarray(Wc[ksl, :].T),
            "wpT": np.ascontiguousarray(Wp[sl, :].T),
            "woT": np.ascontiguousarray(Wo[:, sl].T),
            "bcq": np.ascontiguousarray(bc[sl]),
            "bck": np.ascontiguousarray(bc[ksl]),
            "bp": np.ascontiguousarray(bp[sl]),
        })
    return in_maps


def assemble(per_core_results, bo):
    out = np.zeros((B, T, D), np.float32)
    attn = np.empty((B, H, T, T), np.float32)
    for c in range(NCORES):
        b, g = divmod(c, 2)
        r = per_core_results[c]
        attn[b, g * GH:(g + 1) * GH] = r["attn_out"]
        out[b] += r["yT_out"].T
    out += bo
    return out, attn


def kernel(**inputs):
    global LAST_RESULTS
    x = np.asarray(inputs["x"], np.float32)
    pos = np.asarray(inputs["position_embedding"], np.float32)
    Wc = np.asarray(inputs["Wc"], np.float32)
    bc = np.asarray(inputs["bc"], np.float32)
    Wp = np.asarray(inputs["Wp"], np.float32)
    bp = np.asarray(inputs["bp"], np.float32)
    Wo = np.asarray(inputs["Wo"], np.float32)
    bo = np.asarray(inputs["bo"], np.float32)
    nh = int(np.asarray(inputs.get("num_heads", H)))
    assert nh == H and x.shape == (B, T, D)

    from concourse.bass_utils import run_bass_kernel_spmd

    if "nc" not in _NC_CACHE:
        _NC_CACHE["nc"] = build_bass()
    nc = _NC_CACHE["nc"]

    in_maps = make_in_maps(x, pos, Wc, bc, Wp, bp, Wo)
    res = run_bass_kernel_spmd(
        nc, in_maps, core_ids=list(range(NCORES)), trace=TRACE
    )
    LAST_RESULTS = res
    return assemble(res.results, bo)


# revision 11
# speedup vs baseline: 1.1428x; 1.1428x over previous
"""Disentangled multi-head attention on 8 trn2 NeuronCores.

Sharding: core c -> (batch b = c//2, head-group g = c%2). Each core owns one
batch element and 8 of the 16 heads. Host pre-transposes x/pos and the weight
slices so every on-chip matmul operand is already in [K-on-partitions] layout.

Math (per core, 8 local heads, hd=64):
  qT, kT  = Wcq/Wck slices applied to x      (transposed layout [feat, T])
  kpT     = kT + (Wp slice applied to pos)   (scores = q@(k+pk)^T = q@kp^T)
  S_h     = q_h @ kp_h^T        -> exp(S*scale) -> rowsums (ACT accum) -> attn
  ST_h    = kp_h @ q_h^T        -> exp -> O_h^T = (k_h^T @ expST) * recip
  yT      = Wo_slice^T-matmul over O^T  (host: out[b] = yT0.T + yT1.T + bo)
Outputs per core: attn [8, T, T], yT [D, T].
"""

import numpy as np

B, T, D, H = 4, 1024, 1024, 16
HD = D // H            # 64 head dim
NCORES = 8
GH = H // 2            # 8 heads per core
DL = GH * HD           # 512 local feature dim
P = 128
KD = D // P            # 8 K-subtiles over D
MT = T // P            # 8 token tiles
FQ = DL // P           # 4 local-feature subtiles
NSP = 512              # matmul moving free dim (one fp32 PSUM bank)
NCH = T // NSP         # 2
SCALE = HD ** -0.5

TRACE = False
LAST_RESULTS = None
_NC_CACHE = {}


def build_bass():
    from contextlib import ExitStack

    import concourse.mybir as mybir
    import concourse.tile as tile
    from concourse import bacc
    from concourse.masks import make_identity

    f32 = mybir.dt.float32
    f32r = mybir.dt.float32r
    AF = mybir.ActivationFunctionType
    OP = mybir.AluOpType

    nc = bacc.Bacc("TRN2")

    xT = nc.dram_tensor("xT", [D, T], f32r, kind="ExternalInput")
    posT = nc.dram_tensor("posT", [D, T], f32r, kind="ExternalInput")
    wcqT = nc.dram_tensor("wcqT", [D, DL], f32r, kind="ExternalInput")
    wckT = nc.dram_tensor("wckT", [D, DL], f32r, kind="ExternalInput")
    wpT = nc.dram_tensor("wpT", [D, DL], f32r, kind="ExternalInput")
    woT = nc.dram_tensor("woT", [DL, D], f32r, kind="ExternalInput")
    bcq = nc.dram_tensor("bcq", [DL], f32, kind="ExternalInput")
    bck = nc.dram_tensor("bck", [DL], f32, kind="ExternalInput")
    bp = nc.dram_tensor("bp", [DL], f32, kind="ExternalInput")
    attn_out = nc.dram_tensor("attn_out", [GH, T, T], f32, kind="ExternalOutput")
    yT_out = nc.dram_tensor("yT_out", [D, T], f32, kind="ExternalOutput")

    with tile.TileContext(nc) as tc, ExitStack() as ctx:
        persist = ctx.enter_context(tc.tile_pool(name="persist", bufs=1))
        qT = persist.tile([P, FQ, T], f32r)       # [feat%128, feat//128, T]
        kpT = persist.tile([P, FQ, T], f32r)
        k_nat = persist.tile([P, MT, DL], f32r)   # [t%128, t//128, feat]
        bias_cols = persist.tile([P, 3, FQ], f32)
        ident = persist.tile([P, P], f32)

        nc.sync.dma_start(bias_cols[:, 0], bcq.rearrange("(j p) -> p j", p=P))
        nc.sync.dma_start(bias_cols[:, 1], bck.rearrange("(j p) -> p j", p=P))
        nc.sync.dma_start(bias_cols[:, 2], bp.rearrange("(j p) -> p j", p=P))
        make_identity(nc, ident)

        dramp = ctx.enter_context(tc.tile_pool(name="dramp", bufs=1, space="DRAM"))
        recip_dram = dramp.tile([GH, T], f32)

        xT_v = xT.rearrange("(ks p) t -> p ks t", p=P)
        posT_v = posT.rearrange("(ks p) t -> p ks t", p=P)

        # ---- Phase A: projections (transposed layouts), j-major so the
        # first heads' S matmuls can start while later j's project ----
        with tc.tile_pool(name="wx", bufs=1) as wx, \
             tc.tile_pool(name="pps", bufs=2, space="PSUM") as pps:
            wcq_sb = wx.tile([P, KD, DL], f32r)
            wck_sb = wx.tile([P, KD, DL], f32r)
            wp_sb = wx.tile([P, KD, DL], f32r)
            kT = wx.tile([P, FQ, T], f32)
            wcq_v = wcqT.rearrange("(ks p) f -> p ks f", p=P)
            wck_v = wckT.rearrange("(ks p) f -> p ks f", p=P)
            wp_v = wpT.rearrange("(ks p) f -> p ks f", p=P)

            # per-ks chunked loads so matmuls start after the first chunks
            xcs = [wx.tile([P, KD, NSP], f32r, name=f"xc{n}") for n in range(NCH)]
            pcs = [wx.tile([P, KD, NSP], f32r, name=f"pc{n}") for n in range(NCH)]
            for ks in range(KD):
                nc.sync.dma_start(wck_sb[:, ks], wck_v[:, ks])
            for n in range(NCH):
                for ks in range(KD):
                    nc.sync.dma_start(
                        xcs[n][:, ks], xT_v[:, ks, n * NSP:(n + 1) * NSP])
            for ks in range(KD):
                nc.sync.dma_start(wcq_sb[:, ks], wcq_v[:, ks])
            for ks in range(KD):
                nc.sync.dma_start(wp_sb[:, ks], wp_v[:, ks])
            for n in range(NCH):
                for ks in range(KD):
                    nc.sync.dma_start(
                        pcs[n][:, ks], posT_v[:, ks, n * NSP:(n + 1) * NSP])

            def proj(w_sb, j, xc, dst_evac):
                jsl = slice(j * P, (j + 1) * P)
                ps = pps.tile([P, NSP], f32, tag="pp", name=f"pp{j}")
                for ks in range(KD):
                    nc.tensor.matmul(
                        ps,
                        lhsT=w_sb[:, ks, jsl],
                        rhs=xc[:, ks],
                        start=(ks == 0), stop=(ks == KD - 1),
                    )
                dst_evac(ps)

            for j in range(FQ):
                for n in range(NCH):
                    nsl = slice(n * NSP, (n + 1) * NSP)
                    proj(wck_sb, j, xcs[n], lambda ps, j=j, nsl=nsl:
                         nc.vector.tensor_scalar_add(
                             kT[:, j, nsl], ps, bias_cols[:, 1, j:j + 1]))
                for n in range(NCH):
                    nsl = slice(n * NSP, (n + 1) * NSP)
                    # kpT = (pk + bp) + kT
                    proj(wp_sb, j, pcs[n], lambda ps, j=j, nsl=nsl:
                         nc.vector.scalar_tensor_tensor(
                             out=kpT[:, j, nsl], in0=ps,
                             scalar=bias_cols[:, 2, j:j + 1],
                             in1=kT[:, j, nsl], op0=OP.add, op1=OP.add))
                for n in range(NCH):
                    nsl = slice(n * NSP, (n + 1) * NSP)
                    proj(wcq_sb, j, xcs[n], lambda ps, j=j, nsl=nsl:
                         nc.vector.tensor_scalar_add(
                             qT[:, j, nsl], ps, bias_cols[:, 0, j:j + 1]))

            # k natural layout via PE transposes of kT (kT dies with this pool)
            for ts in range(MT):
                for fj in range(FQ):
                    tpt = pps.tile([P, NSP], f32, tag="pp", name="tpt")
                    nc.tensor.transpose(
                        tpt[:, 0:P], kT[:, fj, ts * P:(ts + 1) * P], ident)
                    nc.vector.tensor_copy(
                        out=k_nat[:, ts, fj * P:(fj + 1) * P], in_=tpt[:, 0:P]
                    )

        # ---- Phase B/C/D ----
        with tc.tile_pool(name="mid", bufs=1) as mid, \
             tc.tile_pool(name="score", bufs=3, space="PSUM") as score, \
             tc.tile_pool(name="ob", bufs=2, space="PSUM") as ob:
            oT = mid.tile([64, GH, T], f32r)
            woT_sb = mid.tile([64, GH, D], f32r)  # [dl%64, head, dout]
            rowsum = mid.tile([P, GH, MT], f32)
            recip = mid.tile([P, GH, MT], f32)
            nc.sync.dma_start(woT_sb, woT.rearrange("(h d) f -> d h f", d=64))

            with tc.tile_pool(name="attp", bufs=4) as attp, \
                 tc.tile_pool(name="estp", bufs=10) as estp, \
                 tc.tile_pool(name="rbc", bufs=2) as rbc, \
                 tc.tile_pool(name="smal", bufs=2) as smal:
                for h in range(GH):
                    base = 64 * (h % 2)
                    j = h // 2
                    qh = qT[base:base + 64, j]     # [64, T]
                    kph = kpT[base:base + 64, j]   # [64, T]

                    # S path: attn output + rowsums
                    for qt in range(MT):
                        ps = score.tile([P, T], f32, tag="sc")
                        for n in range(NCH):
                            nc.tensor.matmul(
                                ps[:, n * NSP:(n + 1) * NSP],
                                lhsT=qh[:, qt * P:(qt + 1) * P],
                                rhs=kph[:, n * NSP:(n + 1) * NSP],
                                start=True, stop=True,
                            )
                        att = attp.tile([P, T], f32, tag="att")
                        nc.scalar.activation(
                            att, ps, AF.Exp, scale=SCALE,
                            accum_out=rowsum[:, h, qt:qt + 1],
                        )
                        nc.vector.reciprocal(
                            recip[:, h, qt:qt + 1], rowsum[:, h, qt:qt + 1]
                        )
                        nc.vector.tensor_scalar_mul(
                            att, att, recip[:, h, qt:qt + 1])
                        nc.sync.dma_start(
                            attn_out[h, qt * P:(qt + 1) * P, :], att)

                    # ST path
                    po = [ob.tile([64, NSP], f32, tag="ob", name=f"po{n}_{h}")
                          for n in range(NCH)]
                    ests = []
                    for kt in range(MT):
                        ps2 = score.tile([P, T], f32, tag="sc")
                        for n in range(NCH):
                            nc.tensor.matmul(
                                ps2[:, n * NSP:(n + 1) * NSP],
                                lhsT=kph[:, kt * P:(kt + 1) * P],
                                rhs=qh[:, n * NSP:(n + 1) * NSP],
                                start=True, stop=True,
                            )
                        est = estp.tile([P, T], f32r, tag="est")
                        nc.scalar.activation(est, ps2, AF.Exp, scale=SCALE)
                        ests.append(est)

                    # recip row [1, T] via PE transpose + DRAM bounce +
                    # partition broadcast (after ST so it never stalls PE)
                    tpr = score.tile([P, T], f32, tag="sc", name="tpr")
                    nc.tensor.transpose(tpr[0:MT, 0:P], recip[:, h, :], ident)
                    rTs = smal.tile([MT, P], f32, tag="rt")
                    nc.vector.tensor_copy(out=rTs, in_=tpr[0:MT, 0:P])
                    nc.sync.dma_start(
                        recip_dram[h].rearrange("(a b) -> a b", a=MT), rTs)
                    rb = rbc.tile([P, T], f32, tag="rb")
                    nc.sync.dma_start(
                        rb, recip_dram[h][None, :].to_broadcast((P, T)))

                    # O accumulation
                    for kt in range(MT):
                        for n in range(NCH):
                            nc.tensor.matmul(
                                po[n],
                                lhsT=k_nat[:, kt, h * HD:(h + 1) * HD],
                                rhs=ests[kt][:, n * NSP:(n + 1) * NSP],
                                start=(kt == 0), stop=(kt == MT - 1),
                            )
                    for n in range(NCH):
                        nsl = slice(n * NSP, (n + 1) * NSP)
                        nc.vector.tensor_tensor(
                            out=oT[:, h, nsl],
                            in0=po[n],
                            in1=rb[0:64, nsl],
                            op=OP.mult,
                        )

            # yT = Wo_slice^T-matmul over O^T
            with tc.tile_pool(name="ysbp", bufs=3) as ysbp:
                for m in range(KD):
                    for n in range(NCH):
                        nsl = slice(n * NSP, (n + 1) * NSP)
                        py = ob.tile([P, NSP], f32, tag="ob")
                        for hh in range(GH):
                            nc.tensor.matmul(
                                py,
                                lhsT=woT_sb[:, hh, m * P:(m + 1) * P],
                                rhs=oT[:, hh, nsl],
                                start=(hh == 0), stop=(hh == GH - 1),
                            )
                        ys = ysbp.tile([P, NSP], f32, tag="ys")
                        nc.vector.tensor_copy(out=ys, in_=py)
                        nc.sync.dma_start(yT_out[m * P:(m + 1) * P, nsl], ys)

    nc.finalize()
    return nc


def make_in_maps(x, pos, Wc, bc, Wp, bp, Wo):
    in_maps = []
    for c in range(NCORES):
        b, g = divmod(c, 2)
        sl = slice(g * DL, (g + 1) * DL)
        ksl = slice(D + g * DL, D + (g + 1) * DL)
        in_maps.append({
            "xT": np.ascontiguousarray(x[b].T),
            "posT": np.ascontiguousarray(pos[b].T),
            "wcqT": np.ascontiguousarray(Wc[sl, :].T),
            "wckT": np.ascontiguousarray(Wc[ksl, :].T),
            "wpT": np.ascontiguousarray(Wp[sl, :].T),
            "woT": np.ascontiguousarray(Wo[:, sl].T),
            "bcq": np.ascontiguousarray(bc[sl]),
            "bck": np.ascontiguousarray(bc[ksl]),
            "bp": np.ascontiguousarray(bp[sl]),
        })
    return in_maps


def assemble(per_core_results, bo):
    out = np.zeros((B, T, D), np.float32)
    attn = np.empty((B, H, T, T), np.float32)
    for c in range(NCORES):
        b, g = divmod(c, 2)
        r = per_core_results[c]
        attn[b, g * GH:(g + 1) * GH] = r["attn_out"]
        out[b] += r["yT_out"].T
    out += bo
    return out, attn


def kernel(**inputs):
    global LAST_RESULTS
    x = np.asarray(inputs["x"], np.float32)
    pos = np.asarray(inputs["position_embedding"], np.float32)
    Wc = np.asarray(inputs["Wc"], np.float32)
    bc = np.asarray(inputs["bc"], np.float32)
    Wp = np.asarray(inputs["Wp"], np.float32)
    bp = np.asarray(inputs["bp"], np.float32)
    Wo = np.asarray(inputs["Wo"], np.float32)
    bo = np.asarray(inputs["bo"], np.float32)
    nh = int(np.asarray(inputs.get("num_heads", H)))
    assert nh == H and x.shape == (B, T, D)

    from concourse.bass_utils import run_bass_kernel_spmd

    if "nc" not in _NC_CACHE:
        _NC_CACHE["nc"] = build_bass()
    nc = _NC_CACHE["nc"]

    in_maps = make_in_maps(x, pos, Wc, bc, Wp, bp, Wo)
    res = run_bass_kernel_spmd(
        nc, in_maps, core_ids=list(range(NCORES)), trace=TRACE
    )
    LAST_RESULTS = res
    return assemble(res.results, bo)


# revision 12
# speedup vs baseline: 1.2301x; 1.0764x over previous
"""Disentangled multi-head attention on 8 trn2 NeuronCores.

Sharding: core c -> (batch b = c//2, head-group g = c%2). Each core owns one
batch element and 8 of the 16 heads. Host pre-transposes x/pos and the weight
slices so every on-chip matmul operand is already in [K-on-partitions] layout.

Math (per core, 8 local heads, hd=64):
  qT, kT  = Wcq/Wck slices applied to x      (transposed layout [feat, T])
  kpT     = kT + (Wp slice applied to pos)   (scores = q@(k+pk)^T = q@kp^T)
  S_h     = q_h @ kp_h^T        -> exp(S*scale) -> rowsums (ACT accum) -> attn
  ST_h    = kp_h @ q_h^T        -> exp -> O_h^T = (k_h^T @ expST) * recip
  yT      = Wo_slice^T-matmul over O^T  (host: out[b] = yT0.T + yT1.T + bo)
Outputs per core: attn [8, T, T], yT [D, T].
"""

import numpy as np

B, T, D, H = 4, 1024, 1024, 16
HD = D // H            # 64 head dim
NCORES = 8
GH = H // 2            # 8 heads per core
DL = GH * HD           # 512 local feature dim
P = 128
KD = D // P            # 8 K-subtiles over D
MT = T // P            # 8 token tiles
FQ = DL // P           # 4 local-feature subtiles
NSP = 512              # matmul moving free dim (one fp32 PSUM bank)
NCH = T // NSP         # 2
SCALE = HD ** -0.5

TRACE = False
LAST_RESULTS = None
_NC_CACHE = {}


def build_bass():
    from contextlib import ExitStack

    import concourse.mybir as mybir
    import concourse.tile as tile
    from concourse import bacc
    from concourse.masks import make_identity

    f32 = mybir.dt.float32
    f32r = mybir.dt.float32r
    AF = mybir.ActivationFunctionType
    OP = mybir.AluOpType

    nc = bacc.Bacc("TRN2")

    xT = nc.dram_tensor("xT", [D, T], f32r, kind="ExternalInput")
    posT = nc.dram_tensor("posT", [D, T], f32r, kind="ExternalInput")
    wcqT = nc.dram_tensor("wcqT", [D, DL], f32r, kind="ExternalInput")
    wckT = nc.dram_tensor("wckT", [D, DL], f32r, kind="ExternalInput")
    wpT = nc.dram_tensor("wpT", [D, DL], f32r, kind="ExternalInput")
    woT = nc.dram_tensor("woT", [DL, D], f32r, kind="ExternalInput")
    bcq = nc.dram_tensor("bcq", [DL], f32, kind="ExternalInput")
    bck = nc.dram_tensor("bck", [DL], f32, kind="ExternalInput")
    bp = nc.dram_tensor("bp", [DL], f32, kind="ExternalInput")
    attn_out = nc.dram_tensor("attn_out", [GH, T, T], f32, kind="ExternalOutput")
    yT_out = nc.dram_tensor("yT_out", [D, T], f32, kind="ExternalOutput")

    with tile.TileContext(nc) as tc, ExitStack() as ctx:
        persist = ctx.enter_context(tc.tile_pool(name="persist", bufs=1))
        qT = persist.tile([P, FQ, T], f32r)       # [feat%128, feat//128, T]
        kpT = persist.tile([P, FQ, T], f32r)
        k_nat = persist.tile([P, MT, DL], f32r)   # [t%128, t//128, feat]
        bias_cols = persist.tile([P, 3, FQ], f32)
        ident = persist.tile([P, P], f32)

        nc.sync.dma_start(bias_cols[:, 0], bcq.rearrange("(j p) -> p j", p=P))
        nc.sync.dma_start(bias_cols[:, 1], bck.rearrange("(j p) -> p j", p=P))
        nc.sync.dma_start(bias_cols[:, 2], bp.rearrange("(j p) -> p j", p=P))
        make_identity(nc, ident)

        dramp = ctx.enter_context(tc.tile_pool(name="dramp", bufs=1, space="DRAM"))
        recip_dram = dramp.tile([GH, T], f32)

        xT_v = xT.rearrange("(ks p) t -> p ks t", p=P)
        posT_v = posT.rearrange("(ks p) t -> p ks t", p=P)

        # ---- Phase A: projections (transposed layouts), j-major so the
        # first heads' S matmuls can start while later j's project ----
        with tc.tile_pool(name="wx", bufs=1) as wx, \
             tc.tile_pool(name="pps", bufs=2, space="PSUM") as pps:
            wcq_sb = wx.tile([P, KD, DL], f32r)
            wck_sb = wx.tile([P, KD, DL], f32r)
            wp_sb = wx.tile([P, KD, DL], f32r)
            kT = wx.tile([P, FQ, T], f32)
            wcq_v = wcqT.rearrange("(ks p) f -> p ks f", p=P)
            wck_v = wckT.rearrange("(ks p) f -> p ks f", p=P)
            wp_v = wpT.rearrange("(ks p) f -> p ks f", p=P)

            # per-ks chunked loads so matmuls start after the first chunks
            xcs = [wx.tile([P, KD, NSP], f32r, name=f"xc{n}") for n in range(NCH)]
            pcs = [wx.tile([P, KD, NSP], f32r, name=f"pc{n}") for n in range(NCH)]
            for ks in range(KD):
                nc.sync.dma_start(wck_sb[:, ks], wck_v[:, ks])
            for n in range(NCH):
                for ks in range(KD):
                    nc.sync.dma_start(
                        xcs[n][:, ks], xT_v[:, ks, n * NSP:(n + 1) * NSP])
            for ks in range(KD):
                nc.sync.dma_start(wcq_sb[:, ks], wcq_v[:, ks])
            for ks in range(KD):
                nc.sync.dma_start(wp_sb[:, ks], wp_v[:, ks])
            for n in range(NCH):
                for ks in range(KD):
                    nc.sync.dma_start(
                        pcs[n][:, ks], posT_v[:, ks, n * NSP:(n + 1) * NSP])

            def proj(w_sb, j, xc, dst_evac):
                jsl = slice(j * P, (j + 1) * P)
                ps = pps.tile([P, NSP], f32, tag="pp", name=f"pp{j}")
                for ks in range(KD):
                    nc.tensor.matmul(
                        ps,
                        lhsT=w_sb[:, ks, jsl],
                        rhs=xc[:, ks],
                        start=(ks == 0), stop=(ks == KD - 1),
                    )
                dst_evac(ps)

            for j in range(FQ):
                for n in range(NCH):
                    nsl = slice(n * NSP, (n + 1) * NSP)
                    proj(wck_sb, j, xcs[n], lambda ps, j=j, nsl=nsl:
                         nc.vector.tensor_scalar_add(
                             kT[:, j, nsl], ps, bias_cols[:, 1, j:j + 1]))
                for n in range(NCH):
                    nsl = slice(n * NSP, (n + 1) * NSP)
                    # kpT = (pk + bp) + kT
                    proj(wp_sb, j, pcs[n], lambda ps, j=j, nsl=nsl:
                         nc.vector.scalar_tensor_tensor(
                             out=kpT[:, j, nsl], in0=ps,
                             scalar=bias_cols[:, 2, j:j + 1],
                             in1=kT[:, j, nsl], op0=OP.add, op1=OP.add))
                # k natural layout via PE transposes (kT dies with this pool);
                # interleaved with projections to keep HAM-visible MMs flowing
                for ts in range(MT):
                    tpt = pps.tile([P, NSP], f32, tag="pp", name="tpt")
                    nc.tensor.transpose(
                        tpt[:, 0:P], kT[:, j, ts * P:(ts + 1) * P], ident)
                    nc.vector.tensor_copy(
                        out=k_nat[:, ts, j * P:(j + 1) * P], in_=tpt[:, 0:P]
                    )
                for n in range(NCH):
                    nsl = slice(n * NSP, (n + 1) * NSP)
                    proj(wcq_sb, j, xcs[n], lambda ps, j=j, nsl=nsl:
                         nc.vector.tensor_scalar_add(
                             qT[:, j, nsl], ps, bias_cols[:, 0, j:j + 1]))

        # ---- Phase B/C/D ----
        with tc.tile_pool(name="mid", bufs=1) as mid, \
             tc.tile_pool(name="score", bufs=3, space="PSUM") as score, \
             tc.tile_pool(name="ob", bufs=2, space="PSUM") as ob:
            oT = mid.tile([64, GH, T], f32r)
            woT_sb = mid.tile([64, GH, D], f32r)  # [dl%64, head, dout]
            rowsum = mid.tile([P, GH, MT], f32)
            recip = mid.tile([P, GH, MT], f32)
            nc.sync.dma_start(woT_sb, woT.rearrange("(h d) f -> d h f", d=64))

            with tc.tile_pool(name="attp", bufs=3) as attp, \
                 tc.tile_pool(name="estp", bufs=12) as estp, \
                 tc.tile_pool(name="rbc", bufs=2) as rbc, \
                 tc.tile_pool(name="smal", bufs=2) as smal:
                for hp in range(GH // 2):
                    h0, h1 = 2 * hp, 2 * hp + 1
                    j = hp
                    qA, qB = qT[0:64, j], qT[64:128, j]       # [64, T]
                    kpA, kpB = kpT[0:64, j], kpT[64:128, j]

                    # S path (pair-interleaved on PE row groups)
                    for qt in range(MT):
                        qsl = slice(qt * P, (qt + 1) * P)
                        psA = score.tile([P, T], f32, tag="sc", name="psA")
                        psB = score.tile([P, T], f32, tag="sc", name="psB")
                        for n in range(NCH):
                            nsl = slice(n * NSP, (n + 1) * NSP)
                            nc.tensor.matmul(psA[:, nsl], lhsT=qA[:, qsl],
                                             rhs=kpA[:, nsl],
                                             start=True, stop=True)
                            nc.tensor.matmul(psB[:, nsl], lhsT=qB[:, qsl],
                                             rhs=kpB[:, nsl],
                                             start=True, stop=True)
                        for h, ps in ((h0, psA), (h1, psB)):
                            att = attp.tile([P, T], f32, tag="att")
                            nc.scalar.activation(
                                att, ps, AF.Exp, scale=SCALE,
                                accum_out=rowsum[:, h, qt:qt + 1],
                            )
                            nc.vector.reciprocal(
                                recip[:, h, qt:qt + 1], rowsum[:, h, qt:qt + 1]
                            )
                            nc.vector.tensor_scalar_mul(
                                att, att, recip[:, h, qt:qt + 1])
                            nc.sync.dma_start(
                                attn_out[h, qt * P:(qt + 1) * P, :], att)

                    # ST path (pair-interleaved); h0's O accumulation inline
                    poA = [ob.tile([64, NSP], f32, tag="ob", name=f"poA{n}")
                           for n in range(NCH)]
                    estsB = []
                    for kt in range(MT):
                        ksl = slice(kt * P, (kt + 1) * P)
                        ps2A = score.tile([P, T], f32, tag="sc", name="ps2A")
                        ps2B = score.tile([P, T], f32, tag="sc", name="ps2B")
                        for n in range(NCH):
                            nsl = slice(n * NSP, (n + 1) * NSP)
                            nc.tensor.matmul(ps2A[:, nsl], lhsT=kpA[:, ksl],
                                             rhs=qA[:, nsl],
                                             start=True, stop=True)
                            nc.tensor.matmul(ps2B[:, nsl], lhsT=kpB[:, ksl],
                                             rhs=qB[:, nsl],
                                             start=True, stop=True)
                        estA = estp.tile([P, T], f32r, tag="est", name="estA")
                        nc.scalar.activation(estA, ps2A, AF.Exp, scale=SCALE)
                        estB = estp.tile([P, T], f32r, tag="est", name="estB")
                        nc.scalar.activation(estB, ps2B, AF.Exp, scale=SCALE)
                        estsB.append(estB)
                        for n in range(NCH):
                            nc.tensor.matmul(
                                poA[n],
                                lhsT=k_nat[:, kt, h0 * HD:(h0 + 1) * HD],
                                rhs=estA[:, n * NSP:(n + 1) * NSP],
                                start=(kt == 0), stop=(kt == MT - 1),
                            )

                    # recip rows for both heads (PE transpose + DRAM bounce)
                    rbs = {}
                    for h in (h0, h1):
                        tpr = score.tile([P, T], f32, tag="sc", name="tpr")
                        nc.tensor.transpose(
                            tpr[0:MT, 0:P], recip[:, h, :], ident)
                        rTs = smal.tile([MT, P], f32, tag="rt")
                        nc.vector.tensor_copy(out=rTs, in_=tpr[0:MT, 0:P])
                        nc.sync.dma_start(
                            recip_dram[h].rearrange("(a b) -> a b", a=MT), rTs)
                        rb = rbc.tile([P, T], f32, tag="rb", name=f"rb{h}")
                        nc.sync.dma_start(
                            rb, recip_dram[h][None, :].to_broadcast((P, T)))
                        rbs[h] = rb

                    for n in range(NCH):
                        nsl = slice(n * NSP, (n + 1) * NSP)
                        nc.vector.tensor_tensor(
                            out=oT[:, h0, nsl], in0=poA[n],
                            in1=rbs[h0][0:64, nsl], op=OP.mult)

                    # h1's O accumulation
                    poB = [ob.tile([64, NSP], f32, tag="ob", name=f"poB{n}")
                           for n in range(NCH)]
                    for kt in range(MT):
                        for n in range(NCH):
                            nc.tensor.matmul(
                                poB[n],
                                lhsT=k_nat[:, kt, h1 * HD:(h1 + 1) * HD],
                                rhs=estsB[kt][:, n * NSP:(n + 1) * NSP],
                                start=(kt == 0), stop=(kt == MT - 1),
                            )
                    for n in range(NCH):
                        nsl = slice(n * NSP, (n + 1) * NSP)
                        nc.vector.tensor_tensor(
                            out=oT[:, h1, nsl], in0=poB[n],
                            in1=rbs[h1][0:64, nsl], op=OP.mult)

            # yT = Wo_slice^T-matmul over O^T
            with tc.tile_pool(name="ysbp", bufs=3) as ysbp:
                for m in range(KD):
                    for n in range(NCH):
                        nsl = slice(n * NSP, (n + 1) * NSP)
                        py = ob.tile([P, NSP], f32, tag="ob")
                        for hh in range(GH):
                            nc.tensor.matmul(
                                py,
                                lhsT=woT_sb[:, hh, m * P:(m + 1) * P],
                                rhs=oT[:, hh, nsl],
                                start=(hh == 0), stop=(hh == GH - 1),
                            )
                        ys = ysbp.tile([P, NSP], f32, tag="ys")
                        nc.vector.tensor_copy(out=ys, in_=py)
                        nc.sync.dma_start(yT_out[m * P:(m + 1) * P, nsl], ys)

    nc.finalize()
    return nc


def make_in_maps(x, pos, Wc, bc, Wp, bp, Wo):
    in_maps = []
    for c in range(NCORES):
        b, g = divmod(c, 2)
        sl = slice(g * DL, (g + 1) * DL)
        ksl = slice(D + g * DL, D + (g + 1) * DL)
        in_maps.append({
            "xT": np.ascontiguousarray(x[b].T),
            "posT": np.ascontiguousarray(pos[b].T),
            "wcqT": np.ascontiguousarray(Wc[sl, :].T),
            "wckT": np.ascontiguousarray(Wc[ksl, :].T),
            "wpT": np.ascontiguousarray(Wp[sl, :].T),
            "woT": np.ascontiguousarray(Wo[:, sl].T),
            "bcq": np.ascontiguousarray(bc[sl]),
            "bck": np.ascontiguousarray(bc[ksl]),
            "bp": np.ascontiguousarray(bp[sl]),
        })
    return in_maps


def assemble(per_core_results, bo):
    out = np.zeros((B, T, D), np.float32)
    attn = np.empty((B, H, T, T), np.float32)
    for c in range(NCORES):
        b, g = divmod(c, 2)
        r = per_core_results[c]
        attn[b, g * GH:(g + 1) * GH] = r["attn_out"]
        out[b] += r["yT_out"].T
    out += bo
    return out, attn


def kernel(**inputs):
    global LAST_RESULTS
    x = np.asarray(inputs["x"], np.float32)
    pos = np.asarray(inputs["position_embedding"], np.float32)
    Wc = np.asarray(inputs["Wc"], np.float32)
    bc = np.asarray(inputs["bc"], np.float32)
    Wp = np.asarray(inputs["Wp"], np.float32)
    bp = np.asarray(inputs["bp"], np.float32)
    Wo = np.asarray(inputs["Wo"], np.float32)
    bo = np.asarray(inputs["bo"], np.float32)
    nh = int(np.asarray(inputs.get("num_heads", H)))
    assert nh == H and x.shape == (B, T, D)

    from concourse.bass_utils import run_bass_kernel_spmd

    if "nc" not in _NC_CACHE:
        _NC_CACHE["nc"] = build_bass()
    nc = _NC_CACHE["nc"]

    in_maps = make_in_maps(x, pos, Wc, bc, Wp, bp, Wo)
    res = run_bass_kernel_spmd(
        nc, in_maps, core_ids=list(range(NCORES)), trace=TRACE
    )
    LAST_RESULTS = res
    return assemble(res.results, bo)


# revision 13
# speedup vs baseline: 1.3302x; 1.0813x over previous
"""Disentangled multi-head attention on 8 trn2 NeuronCores.

Sharding: core c -> (batch b = c//2, head-group g = c%2). Each core owns one
batch element and 8 of the 16 heads. Host pre-transposes x/pos and the weight
slices so every on-chip matmul operand is already in [K-on-partitions] layout.

Math (per core, 8 local heads, hd=64):
  qT, kT  = Wcq/Wck slices applied to x      (transposed layout [feat, T])
  kpT     = kT + (Wp slice applied to pos)   (scores = q@(k+pk)^T = q@kp^T)
  S_h     = q_h @ kp_h^T        -> exp(S*scale) -> rowsums (ACT accum) -> attn
  ST_h    = kp_h @ q_h^T        -> exp -> O_h^T = (k_h^T @ expST) * recip
  yT      = Wo_slice^T-matmul over O^T  (host: out[b] = yT0.T + yT1.T + bo)
Outputs per core: attn [8, T, T], yT [D, T].
"""

import numpy as np

B, T, D, H = 4, 1024, 1024, 16
HD = D // H            # 64 head dim
NCORES = 8
GH = H // 2            # 8 heads per core
DL = GH * HD           # 512 local feature dim
P = 128
KD = D // P            # 8 K-subtiles over D
MT = T // P            # 8 token tiles
FQ = DL // P           # 4 local-feature subtiles
NSP = 512              # matmul moving free dim (one fp32 PSUM bank)
NCH = T // NSP         # 2
SCALE = HD ** -0.5

TRACE = False
LAST_RESULTS = None
_NC_CACHE = {}


def build_bass():
    from contextlib import ExitStack

    import concourse.mybir as mybir
    import concourse.tile as tile
    from concourse import bacc
    from concourse.masks import make_identity

    f32 = mybir.dt.float32
    f32r = mybir.dt.float32r
    AF = mybir.ActivationFunctionType
    OP = mybir.AluOpType

    nc = bacc.Bacc("TRN2")

    xT = nc.dram_tensor("xT", [D, T], f32r, kind="ExternalInput")
    posT = nc.dram_tensor("posT", [D, T], f32r, kind="ExternalInput")
    wcqT = nc.dram_tensor("wcqT", [D, DL], f32r, kind="ExternalInput")
    wckT = nc.dram_tensor("wckT", [D, DL], f32r, kind="ExternalInput")
    wpT = nc.dram_tensor("wpT", [D, DL], f32r, kind="ExternalInput")
    woT = nc.dram_tensor("woT", [DL, D], f32r, kind="ExternalInput")
    bcq = nc.dram_tensor("bcq", [DL], f32, kind="ExternalInput")
    bck = nc.dram_tensor("bck", [DL], f32, kind="ExternalInput")
    bp = nc.dram_tensor("bp", [DL], f32, kind="ExternalInput")
    attn_out = nc.dram_tensor("attn_out", [GH, T, T], f32, kind="ExternalOutput")
    yT_out = nc.dram_tensor("yT_out", [D, T], f32, kind="ExternalOutput")

    with tile.TileContext(nc) as tc, ExitStack() as ctx:
        persist = ctx.enter_context(tc.tile_pool(name="persist", bufs=1))
        qT = persist.tile([P, FQ, T], f32r)       # [feat%128, feat//128, T]
        kpT = persist.tile([P, FQ, T], f32r)
        k_nat = persist.tile([P, MT, DL], f32r)   # [t%128, t//128, feat]
        bias_cols = persist.tile([P, 3, FQ], f32)
        ident = persist.tile([P, P], f32)

        nc.sync.dma_start(bias_cols[:, 0], bcq.rearrange("(j p) -> p j", p=P))
        nc.sync.dma_start(bias_cols[:, 1], bck.rearrange("(j p) -> p j", p=P))
        nc.sync.dma_start(bias_cols[:, 2], bp.rearrange("(j p) -> p j", p=P))
        make_identity(nc, ident)

        dramp = ctx.enter_context(tc.tile_pool(name="dramp", bufs=1, space="DRAM"))
        recip_dram = dramp.tile([GH, T], f32)

        xT_v = xT.rearrange("(ks p) t -> p ks t", p=P)
        posT_v = posT.rearrange("(ks p) t -> p ks t", p=P)

        # ---- Phase A: projections (transposed layouts), j-major so the
        # first heads' S matmuls can start while later j's project ----
        with tc.tile_pool(name="wx", bufs=1) as wx, \
             tc.tile_pool(name="pps", bufs=2, space="PSUM") as pps:
            wcq_sb = wx.tile([P, KD, DL], f32r)
            wck_sb = wx.tile([P, KD, DL], f32r)
            wp_sb = wx.tile([P, KD, DL], f32r)
            kT = wx.tile([P, FQ, T], f32)
            wcq_v = wcqT.rearrange("(ks p) f -> p ks f", p=P)
            wck_v = wckT.rearrange("(ks p) f -> p ks f", p=P)
            wp_v = wpT.rearrange("(ks p) f -> p ks f", p=P)

            # per-ks chunked loads, ordered to match consumption:
            # k-projs (wck+xc), kp-projs (wp+pc), q-projs (wcq)
            xcs = [wx.tile([P, KD, NSP], f32r, name=f"xc{n}") for n in range(NCH)]
            pcs = [wx.tile([P, KD, NSP], f32r, name=f"pc{n}") for n in range(NCH)]
            for ks in range(KD):
                nc.sync.dma_start(wck_sb[:, ks], wck_v[:, ks])
            for n in range(NCH):
                for ks in range(KD):
                    nc.sync.dma_start(
                        xcs[n][:, ks], xT_v[:, ks, n * NSP:(n + 1) * NSP])
            for ks in range(KD):
                nc.sync.dma_start(wp_sb[:, ks], wp_v[:, ks])
            for n in range(NCH):
                for ks in range(KD):
                    nc.sync.dma_start(
                        pcs[n][:, ks], posT_v[:, ks, n * NSP:(n + 1) * NSP])
            for ks in range(KD):
                nc.sync.dma_start(wcq_sb[:, ks], wcq_v[:, ks])

            def proj(w_sb, j, xc, dst_evac):
                jsl = slice(j * P, (j + 1) * P)
                ps = pps.tile([P, NSP], f32, tag="pp", name=f"pp{j}")
                for ks in range(KD):
                    nc.tensor.matmul(
                        ps,
                        lhsT=w_sb[:, ks, jsl],
                        rhs=xc[:, ks],
                        start=(ks == 0), stop=(ks == KD - 1),
                    )
                dst_evac(ps)

            for n in range(NCH):
                for j in range(FQ):
                    nsl = slice(n * NSP, (n + 1) * NSP)
                    proj(wck_sb, j, xcs[n], lambda ps, j=j, nsl=nsl:
                         nc.vector.tensor_scalar_add(
                             kT[:, j, nsl], ps, bias_cols[:, 1, j:j + 1]))
            for n in range(NCH):
                for j in range(FQ):
                    nsl = slice(n * NSP, (n + 1) * NSP)
                    # kpT = (pk + bp) + kT
                    proj(wp_sb, j, pcs[n], lambda ps, j=j, nsl=nsl:
                         nc.vector.scalar_tensor_tensor(
                             out=kpT[:, j, nsl], in0=ps,
                             scalar=bias_cols[:, 2, j:j + 1],
                             in1=kT[:, j, nsl], op0=OP.add, op1=OP.add))
                    # a few HAM-invisible PE transposes between projections
                    # (k natural layout; kT dies with this pool)
                    fj = 2 * j + n
                    for ts in range(MT // 2):
                        tso = (MT // 2) * (fj % 2)
                        fjj = fj // 2
                        tpt = pps.tile([P, NSP], f32, tag="pp", name="tpt")
                        nc.tensor.transpose(
                            tpt[:, 0:P],
                            kT[:, fjj, (ts + tso) * P:(ts + tso + 1) * P],
                            ident)
                        nc.vector.tensor_copy(
                            out=k_nat[:, ts + tso, fjj * P:(fjj + 1) * P],
                            in_=tpt[:, 0:P])
            for n in range(NCH):
                for j in range(FQ):
                    nsl = slice(n * NSP, (n + 1) * NSP)
                    proj(wcq_sb, j, xcs[n], lambda ps, j=j, nsl=nsl:
                         nc.vector.tensor_scalar_add(
                             qT[:, j, nsl], ps, bias_cols[:, 0, j:j + 1]))

        # ---- Phase B/C/D ----
        with tc.tile_pool(name="mid", bufs=1) as mid, \
             tc.tile_pool(name="score", bufs=3, space="PSUM") as score, \
             tc.tile_pool(name="ob", bufs=2, space="PSUM") as ob:
            oT = mid.tile([64, GH, T], f32r)
            woT_sb = mid.tile([64, GH, D], f32r)  # [dl%64, head, dout]
            rowsum = mid.tile([P, GH, MT], f32)
            recip = mid.tile([P, GH, MT], f32)
            nc.sync.dma_start(woT_sb, woT.rearrange("(h d) f -> d h f", d=64))

            with tc.tile_pool(name="attp", bufs=3) as attp, \
                 tc.tile_pool(name="estp", bufs=12) as estp, \
                 tc.tile_pool(name="rbc", bufs=2) as rbc, \
                 tc.tile_pool(name="smal", bufs=2) as smal:
                for hp in range(GH // 2):
                    h0, h1 = 2 * hp, 2 * hp + 1
                    j = hp
                    qA, qB = qT[0:64, j], qT[64:128, j]       # [64, T]
                    kpA, kpB = kpT[0:64, j], kpT[64:128, j]

                    # S path (pair-interleaved on PE row groups)
                    for qt in range(MT):
                        qsl = slice(qt * P, (qt + 1) * P)
                        psA = score.tile([P, T], f32, tag="sc", name="psA")
                        psB = score.tile([P, T], f32, tag="sc", name="psB")
                        for n in range(NCH):
                            nsl = slice(n * NSP, (n + 1) * NSP)
                            nc.tensor.matmul(psA[:, nsl], lhsT=qA[:, qsl],
                                             rhs=kpA[:, nsl],
                                             start=True, stop=True)
                            nc.tensor.matmul(psB[:, nsl], lhsT=qB[:, qsl],
                                             rhs=kpB[:, nsl],
                                             start=True, stop=True)
                        for h, ps in ((h0, psA), (h1, psB)):
                            att = attp.tile([P, T], f32, tag="att")
                            nc.scalar.activation(
                                att, ps, AF.Exp, scale=SCALE,
                                accum_out=rowsum[:, h, qt:qt + 1],
                            )
                            nc.vector.reciprocal(
                                recip[:, h, qt:qt + 1], rowsum[:, h, qt:qt + 1]
                            )
                            nc.vector.tensor_scalar_mul(
                                att, att, recip[:, h, qt:qt + 1])
                            nc.sync.dma_start(
                                attn_out[h, qt * P:(qt + 1) * P, :], att)

                    # ST path (pair-interleaved); h0's O accumulation inline
                    poA = [ob.tile([64, NSP], f32, tag="ob", name=f"poA{n}")
                           for n in range(NCH)]
                    estsB = []
                    for kt in range(MT):
                        ksl = slice(kt * P, (kt + 1) * P)
                        ps2A = score.tile([P, T], f32, tag="sc", name="ps2A")
                        ps2B = score.tile([P, T], f32, tag="sc", name="ps2B")
                        for n in range(NCH):
                            nsl = slice(n * NSP, (n + 1) * NSP)
                            nc.tensor.matmul(ps2A[:, nsl], lhsT=kpA[:, ksl],
                                             rhs=qA[:, nsl],
                                             start=True, stop=True)
                            nc.tensor.matmul(ps2B[:, nsl], lhsT=kpB[:, ksl],
                                             rhs=qB[:, nsl],
                                             start=True, stop=True)
                        estA = estp.tile([P, T], f32r, tag="est", name="estA")
                        nc.scalar.activation(estA, ps2A, AF.Exp, scale=SCALE)
                        estB = estp.tile([P, T], f32r, tag="est", name="estB")
                        nc.scalar.activation(estB, ps2B, AF.Exp, scale=SCALE)
                        estsB.append(estB)
                        for n in range(NCH):
                            nc.tensor.matmul(
                                poA[n],
                                lhsT=k_nat[:, kt, h0 * HD:(h0 + 1) * HD],
                                rhs=estA[:, n * NSP:(n + 1) * NSP],
                                start=(kt == 0), stop=(kt == MT - 1),
                            )

                    # recip rows for both heads (PE transpose + DRAM bounce)
                    rbs = {}
                    for h in (h0, h1):
                        tpr = score.tile([P, T], f32, tag="sc", name="tpr")
                        nc.tensor.transpose(
                            tpr[0:MT, 0:P], recip[:, h, :], ident)
                        rTs = smal.tile([MT, P], f32, tag="rt")
                        nc.vector.tensor_copy(out=rTs, in_=tpr[0:MT, 0:P])
                        nc.sync.dma_start(
                            recip_dram[h].rearrange("(a b) -> a b", a=MT), rTs)
                        rb = rbc.tile([P, T], f32, tag="rb", name=f"rb{h}")
                        nc.sync.dma_start(
                            rb, recip_dram[h][None, :].to_broadcast((P, T)))
                        rbs[h] = rb

                    for n in range(NCH):
                        nsl = slice(n * NSP, (n + 1) * NSP)
                        nc.vector.tensor_tensor(
                            out=oT[:, h0, nsl], in0=poA[n],
                            in1=rbs[h0][0:64, nsl], op=OP.mult)

                    # h1's O accumulation
                    poB = [ob.tile([64, NSP], f32, tag="ob", name=f"poB{n}")
                           for n in range(NCH)]
                    for kt in range(MT):
                        for n in range(NCH):
                            nc.tensor.matmul(
                                poB[n],
                                lhsT=k_nat[:, kt, h1 * HD:(h1 + 1) * HD],
                                rhs=estsB[kt][:, n * NSP:(n + 1) * NSP],
                                start=(kt == 0), stop=(kt == MT - 1),
                            )
                    for n in range(NCH):
                        nsl = slice(n * NSP, (n + 1) * NSP)
                        nc.vector.tensor_tensor(
                            out=oT[:, h1, nsl], in0=poB[n],
                            in1=rbs[h1][0:64, nsl], op=OP.mult)

            # yT = Wo_slice^T-matmul over O^T
            with tc.tile_pool(name="ysbp", bufs=3) as ysbp:
                for m in range(KD):
                    for n in range(NCH):
                        nsl = slice(n * NSP, (n + 1) * NSP)
                        py = ob.tile([P, NSP], f32, tag="ob")
                        for hh in range(GH):
                            nc.tensor.matmul(
                                py,
                                lhsT=woT_sb[:, hh, m * P:(m + 1) * P],
                                rhs=oT[:, hh, nsl],
                                start=(hh == 0), stop=(hh == GH - 1),
                            )
                        ys = ysbp.tile([P, NSP], f32, tag="ys")
                        nc.vector.tensor_copy(out=ys, in_=py)
                        nc.sync.dma_start(yT_out[m * P:(m + 1) * P, nsl], ys)

    nc.finalize()
    return nc


def make_in_maps(x, pos, Wc, bc, Wp, bp, Wo):
    in_maps = []
    for c in range(NCORES):
        b, g = divmod(c, 2)
        sl = slice(g * DL, (g + 1) * DL)
        ksl = slice(D + g * DL, D + (g + 1) * DL)
        in_maps.append({
            "xT": np.ascontiguousarray(x[b].T),
            "posT": np.ascontiguousarray(pos[b].T),
            "wcqT": np.ascontiguousarray(Wc[sl, :].T),
            "wckT": np.ascontiguousarray(Wc[ksl, :].T),
            "wpT": np.ascontiguousarray(Wp[sl, :].T),
            "woT": np.ascontiguousarray(Wo[:, sl].T),
            "bcq": np.ascontiguousarray(bc[sl]),
            "bck": np.ascontiguousarray(bc[ksl]),
            "bp": np.ascontiguousarray(bp[sl]),
        })
    return in_maps


def assemble(per_core_results, bo):
    out = np.zeros((B, T, D), np.float32)
    attn = np.empty((B, H, T, T), np.float32)
    for c in range(NCORES):
        b, g = divmod(c, 2)
        r = per_core_results[c]
        attn[b, g * GH:(g + 1) * GH] = r["attn_out"]
        out[b] += r["yT_out"].T
    out += bo
    return out, attn


def kernel(**inputs):
    global LAST_RESULTS
    x = np.asarray(inputs["x"], np.float32)
    pos = np.asarray(inputs["position_embedding"], np.float32)
    Wc = np.asarray(inputs["Wc"], np.float32)
    bc = np.asarray(inputs["bc"], np.float32)
    Wp = np.asarray(inputs["Wp"], np.float32)
    bp = np.asarray(inputs["bp"], np.float32)
    Wo = np.asarray(inputs["Wo"], np.float32)
    bo = np.asarray(inputs["bo"], np.float32)
    nh = int(np.asarray(inputs.get("num_heads", H)))
    assert nh == H and x.shape == (B, T, D)

    from concourse.bass_utils import run_bass_kernel_spmd

    if "nc" not in _NC_CACHE:
        _NC_CACHE["nc"] = build_bass()
    nc = _NC_CACHE["nc"]

    in_maps = make_in_maps(x, pos, Wc, bc, Wp, bp, Wo)
    res = run_bass_kernel_spmd(
        nc, in_maps, core_ids=list(range(NCORES)), trace=TRACE
    )
    LAST_RESULTS = res
    return assemble(res.results, bo)
